# revision 3
# baseline (speedup 1.0000x reference)
"""BN-LSTM (2-layer, Cooijmans) TRN2 Bass kernel v3 — 8-way batch-parallel.

Key differences vs the v2 baseline (see kernel.py docstring):
- ONE fused loop: no separate input-projection phase. wi0[t] = x_t @ w_ih0 is
  recomputed per step into PSUM (cheap matmul) instead of a 512KB/step DRAM
  round trip; its BN stats ride the same per-step collective as wh0's.
- AllReduce of raw [sum, sumsq] (16KB) instead of AllGather of bn_stats
  records (36KB in / 295KB readback per step) — kills the dominant DMA cost.
- rsqrt via the quake bit-trick + 2 Newton iterations on the Vector engine:
  the Scalar engine runs ONLY sigmoid/tanh/identity (one activation-table
  set) — no ~2.7us table reloads on the critical path.
- wi1[t] stays in PSUM across one step (no scalar copy, no SBUF staging).
- gpsimd queue carries only collectives + their bounce DMAs.
"""

import time
from contextlib import ExitStack

import numpy as np

import concourse.bass as bass
import concourse.mybir as mybir
import concourse.bacc as bacc
import concourse.tile as tile
from concourse.bass_utils import run_bass_kernel_spmd

B, T, I, H, O = 2048, 152, 75, 128, 256
NCORES = 8
BL = B // NCORES
G = 4
EPS = 1e-5

fp32 = mybir.dt.float32
i32 = mybir.dt.int32
AF = mybir.ActivationFunctionType
ALU = mybir.AluOpType
AX = mybir.AxisListType
RG = [list(range(NCORES))]
MAGIC = 0x5F3759DF


def _build_v3(repeats=1, local_stats=False, nr_iters=1, use_f32r=False,
              use_bf16=True):
    nc = bacc.Bacc("TRN2", target_bir_lowering=False, debug=False,
                   num_devices=NCORES)

    mmdt = mybir.dt.bfloat16 if use_bf16 else fp32
    xT = nc.dram_tensor("xT", [I, T, BL], mmdt, kind="ExternalInput").ap()
    wih0 = nc.dram_tensor("wih0", [I, G * H], mmdt, kind="ExternalInput").ap()
    whh0 = nc.dram_tensor("whh0", [H, G * H], mmdt, kind="ExternalInput").ap()
    wih1 = nc.dram_tensor("wih1", [H, G * H], mmdt, kind="ExternalInput").ap()
    whh1 = nc.dram_tensor("whh1", [H, G * H], mmdt, kind="ExternalInput").ap()
    fcwT = nc.dram_tensor("fcwT", [H, O], mmdt, kind="ExternalInput").ap()
    gcat16 = nc.dram_tensor("gcat16", [H, 16], fp32, kind="ExternalInput").ap()
    bccat = nc.dram_tensor("bccat", [H, 8], fp32, kind="ExternalInput").ap()
    gc2 = nc.dram_tensor("gc2", [H, 2], fp32, kind="ExternalInput").ap()
    bec2 = nc.dram_tensor("bec2", [H, 2], fp32, kind="ExternalInput").ap()
    y = nc.dram_tensor("y", [BL, O], fp32, kind="ExternalOutput").ap()

    def mmcast(ap):
        return ap.bitcast(mybir.dt.float32r) if use_f32r else ap

    with tile.TileContext(nc) as tc, ExitStack() as ctx:
        sb = ctx.enter_context(tc.tile_pool(name="sb", bufs=1))
        loop = ctx.enter_context(tc.tile_pool(name="loop", bufs=3))
        psum = ctx.enter_context(tc.tile_pool(name="psum", bufs=1,
                                              space="PSUM"))
        dram = ctx.enter_context(tc.tile_pool(name="dram", bufs=2,
                                              space="DRAM"))

        def load(ap_in, shape, name, dt=fp32):
            t_ = sb.tile(shape, dt, name=name)
            nc.sync.dma_start(t_[:], ap_in[:])
            return t_

        wih0_sb = load(wih0, [I, G * H], "wih0_sb", mmdt)
        whh0_sb = load(whh0, [H, G * H], "whh0_sb", mmdt)
        wih1_sb = load(wih1, [H, G * H], "wih1_sb", mmdt)
        whh1_sb = load(whh1, [H, G * H], "whh1_sb", mmdt)
        fcw_sb = load(fcwT, [H, O], "fcw_sb", mmdt)
        gcat = load(gcat16, [H, 16], "gcat_sb")      # [ghh0|gih0|ghh1|gih1]
        bcc = load(bccat, [H, 8], "bcc_sb")          # [bhh0+bih0+b0 | ..1]
        gc2_sb = load(gc2, [H, 2], "gc2_sb")
        bec2_sb = load(bec2, [H, 2], "bec2_sb")

        magic_t = sb.tile([H, 16], i32, name="magic_t")
        nc.vector.memset(magic_t[:], MAGIC)

        # persistent stats-record tile: 16 groups x 6 bn_stats fields
        # group layout: [wh0 0:4 | wi0 4:8 | wh1 8:12 | wi1 12:16]
        stA = sb.tile([H, 16, 6], fp32, name="stA")
        nc.vector.memset(stA[:], 0.0)

        def quake(dst, src, n, eng=nc.vector, nr=nr_iters):
            """dst = 1/sqrt(src) elementwise, (H, n) tiles, DVE-only."""
            di = dst[:].bitcast(i32)
            eng.tensor_scalar(di, src.bitcast(i32), 1, None,
                              op0=ALU.logical_shift_right)
            eng.tensor_tensor(di, magic_t[:, 0:n].bitcast(i32), di,
                              op=ALU.subtract)
            for _ in range(nr):
                yy = loop.tile([H, n], fp32, tag=f"qk_yy{n}", name="yy")
                eng.tensor_tensor(yy[:], dst[:], dst[:], op=ALU.mult)
                eng.tensor_tensor(yy[:], src, yy[:], op=ALU.mult)
                eng.tensor_scalar(yy[:], yy[:], -0.5, 1.5, op0=ALU.mult,
                                  op1=ALU.add)
                eng.tensor_tensor(dst[:], dst[:], yy[:], op=ALU.mult)

        def converts(st_view, out_tile, n, eng=nc.gpsimd):
            """bn_stats records (H, n, 6) -> out (H, 2n) = [128*(me+mo) sums,
            sumsq]."""
            me = st_view[:, :, 1]
            mo = st_view[:, :, 4]
            s1 = loop.tile([H, n], fp32, tag=f"cv_s1{n}", name="s1")
            eng.tensor_tensor(s1[:], me, mo, op=ALU.add)
            nc.vector.tensor_scalar(out_tile[:, 0:n], s1[:], float(BL // 2),
                                    None, op0=ALU.mult)
            q1 = loop.tile([H, n], fp32, tag=f"cv_q1{n}", name="q1")
            eng.tensor_tensor(q1[:], me, me, op=ALU.mult)
            q2 = loop.tile([H, n], fp32, tag=f"cv_q2{n}", name="q2")
            eng.tensor_tensor(q2[:], mo, mo, op=ALU.mult)
            eng.tensor_tensor(q1[:], q1[:], q2[:], op=ALU.add)
            m2s = loop.tile([H, n], fp32, tag=f"cv_m2{n}", name="m2s")
            eng.tensor_tensor(m2s[:], st_view[:, :, 2], st_view[:, :, 5],
                              op=ALU.add)
            nc.vector.scalar_tensor_tensor(
                out_tile[:, n:2 * n], in0=q1[:], scalar=float(BL // 2),
                in1=m2s[:], op0=ALU.mult, op1=ALU.add)

        def allreduce(in_tile, n, tag):
            """AllGather + local 8-way tree add -> SBUF tile (H, n)."""
            din = dram.tile([H, n], fp32, tag=f"{tag}_in", name=f"{tag}_in")
            nc.gpsimd.dma_start(din[:], in_tile[:])
            if local_stats:
                res = loop.tile([H, n], fp32, tag=f"{tag}_res",
                                name=f"{tag}_res")
                nc.gpsimd.dma_start(res[:], din[:])
                # fake the x8 sum so magnitudes match the collective path
                nc.vector.tensor_scalar(res[:], res[:], 8.0, None,
                                        op0=ALU.mult)
                return res
            dout = dram.tile([NCORES * H, n], fp32, tag=f"{tag}_out",
                             addr_space="Shared", name=f"{tag}_out")
            nc.gpsimd.collective_compute(
                "AllGather", ALU.bypass, replica_groups=RG,
                ins=[din[:]], outs=[dout[:]])
            rb = loop.tile([H, NCORES, n], fp32, tag=f"{tag}_rb",
                           name=f"{tag}_rb")
            nc.gpsimd.dma_start(
                rb[:], dout[:].rearrange("(r p) s -> p r s", r=NCORES))
            v4 = rb[:].rearrange("p (a b) s -> p a b s", a=2)
            r4 = loop.tile([H, 4, n], fp32, tag=f"{tag}_r4", name=f"{tag}_r4")
            nc.gpsimd.tensor_tensor(r4[:], v4[:, 0], v4[:, 1], op=ALU.add)
            r4v = r4[:].rearrange("p (a b) s -> p a b s", a=2)
            r2 = loop.tile([H, 2, n], fp32, tag=f"{tag}_r2", name=f"{tag}_r2")
            nc.gpsimd.tensor_tensor(r2[:], r4v[:, 0], r4v[:, 1], op=ALU.add)
            res = loop.tile([H, n], fp32, tag=f"{tag}_res", name=f"{tag}_res")
            nc.vector.tensor_tensor(res[:], r2[:, 0], r2[:, 1], op=ALU.add)
            return res

        h0 = loop.tile([H, BL], mmdt, tag="h0", name="h0")
        h1 = loop.tile([H, BL], mmdt, tag="h1", name="h1")
        cc = loop.tile([H, 2, BL], fp32, tag="cc", name="cc")
        for t_ in (h0, h1):
            nc.vector.memset(t_[:], 0.0)
        nc.vector.memset(cc[:], 0.0)

        scale_mean = 1.0 / (B // (BL // 2))  # AR(sum)/2048 with the 128 factor
        # arin[:,0:16] already scaled by BL/2=128, so mean = AR/(2048/1)*... :
        # arin holds 128*(me+mo) = sum. So mean factor is 1/2048.
        MEAN_SCALE = 1.0 / B

        h1_fin = None
        for _rep in range(repeats):
            if _rep > 0:
                for t_ in (h0, h1):
                    nc.vector.memset(t_[:], 0.0)
                nc.vector.memset(cc[:], 0.0)
                nc.vector.memset(stA[:], 0.0)

            for t in range(T + 1):
                has0 = t < T
                has1 = t >= 1

                # ---------------- matmuls + stats ----------------
                pw0 = pwi0 = pw1 = None
                if has0:
                    xt = loop.tile([I, BL], mmdt, tag="xt", bufs=4, name="xt")
                    nc.sync.dma_start(xt[:], xT[:, t, :])
                    pwi0 = psum.tile([H, G, BL], fp32, tag="wi0", name="pwi0")
                    for q in range(G):
                        nc.tensor.matmul(
                            pwi0[:, q, :],
                            mmcast(wih0_sb[:, q * H:(q + 1) * H]),
                            mmcast(xt[:]), start=True, stop=True)
                    pw0 = psum.tile([H, G, BL], fp32, tag="g0", name="pw0")
                    for q in range(G):
                        nc.tensor.matmul(
                            pw0[:, q, :],
                            mmcast(whh0_sb[:, q * H:(q + 1) * H]),
                            mmcast(h0[:]), start=True, stop=True)
                    for q in range(G):
                        nc.vector.bn_stats(stA[:, q, :], pw0[:, q, :])
                        nc.vector.bn_stats(stA[:, 4 + q, :], pwi0[:, q, :])
                if has1:
                    pw1 = psum.tile([H, G, BL], fp32, tag="g1", name="pw1")
                    for q in range(G):
                        nc.tensor.matmul(
                            pw1[:, q, :],
                            mmcast(whh1_sb[:, q * H:(q + 1) * H]),
                            mmcast(h1[:]), start=True, stop=True)
                    for q in range(G):
                        nc.vector.bn_stats(stA[:, 8 + q, :], pw1[:, q, :])
                    # stA[:, 12:16] (wi1[t-1]) written at tail of step t-1

                # ---------------- AR-A ----------------
                arin = loop.tile([H, 32], fp32, tag="arin", name="arin")
                converts(stA[:], arin, 16)
                ra = allreduce(arin, 32, "arA")
                if has0:
                    # PSUM->SBUF copy runs during the collective wait
                    wi0sb = loop.tile([H, G, BL], fp32, tag="wi0sb",
                                      name="wi0sb")
                    nc.scalar.copy(wi0sb[:], pwi0[:])
                if has1:
                    st_wi1sb = loop.tile([H, G, BL], fp32, tag="wi1sb",
                                         name="wi1sb")
                    nc.scalar.copy(st_wi1sb[:], st_pwm[:])

                # ---------------- post-AR-A scales ----------------
                mv = loop.tile([H, 32], fp32, tag="mv", name="mv")
                nc.vector.tensor_scalar(mv[:, 0:16], ra[:, 0:16], MEAN_SCALE,
                                        None, op0=ALU.mult)
                # E2 + eps in one op; var+eps = (E2+eps) - mean^2
                nc.vector.tensor_scalar(mv[:, 16:32], ra[:, 16:32], MEAN_SCALE,
                                        EPS, op0=ALU.mult, op1=ALU.add)
                m2 = loop.tile([H, 16], fp32, tag="m2", name="m2")
                nc.vector.tensor_tensor(m2[:], mv[:, 0:16], mv[:, 0:16],
                                        op=ALU.mult)
                vep = loop.tile([H, 16], fp32, tag="vep", name="vep")
                nc.vector.scalar_tensor_tensor(
                    vep[:], in0=m2[:], scalar=-1.0, in1=mv[:, 16:32],
                    op0=ALU.mult, op1=ALU.add)
                rsq = loop.tile([H, 16], fp32, tag="rsq", name="rsq")
                quake(rsq, vep[:], 16)
                s16 = loop.tile([H, 16], fp32, tag="s16", name="s16")
                nc.vector.tensor_tensor(s16[:], rsq[:], gcat[:], op=ALU.mult)
                ms = loop.tile([H, 16], fp32, tag="ms", name="ms")
                nc.vector.tensor_tensor(ms[:], mv[:, 0:16], s16[:],
                                        op=ALU.mult)
                # layered views: (H, 2, 4): [:,0,:]=layer0, [:,1,:]=layer1
                sv = s16[:].rearrange("p (l k g) -> p l k g", l=2, k=2)
                msv = ms[:].rearrange("p (l k g) -> p l k g", l=2, k=2)
                is8 = loop.tile([H, 2, G], fp32, tag="is8", name="is8")
                nc.vector.reciprocal(is8[:], sv[:, :, 1, :])
                r8 = loop.tile([H, 2, G], fp32, tag="r8", name="r8")
                nc.vector.tensor_tensor(r8[:], sv[:, :, 0, :], is8[:],
                                        op=ALU.mult)
                v8 = loop.tile([H, 2, G], fp32, tag="v8", name="v8")
                bcv = bcc[:].rearrange("p (l g) -> p l g", l=2)
                nc.vector.tensor_tensor(v8[:], bcv, msv[:, :, 0, :],
                                        op=ALU.subtract)
                nc.vector.tensor_tensor(v8[:], v8[:], msv[:, :, 1, :],
                                        op=ALU.subtract)

                # ---------------- gates ----------------
                FNS = (AF.Sigmoid, AF.Sigmoid, AF.Sigmoid, AF.Tanh)
                ga0 = ga1 = None
                if has0:
                    for q in range(G):
                        nc.vector.scalar_tensor_tensor(
                            pw0[:, q, :], in0=pw0[:, q, :],
                            scalar=r8[:, 0, q:q + 1], in1=wi0sb[:, q, :],
                            op0=ALU.mult, op1=ALU.add)
                    ga0 = loop.tile([H, G, BL], fp32, tag="ga0", name="ga0")
                    for q in range(G):
                        nc.scalar.activation(ga0[:, q, :], pw0[:, q, :],
                                             FNS[q], bias=v8[:, 0, q:q + 1],
                                             scale=sv[:, 0, 1, q:q + 1])
                if has1:
                    for q in range(G):
                        nc.vector.scalar_tensor_tensor(
                            pw1[:, q, :], in0=pw1[:, q, :],
                            scalar=r8[:, 1, q:q + 1], in1=st_wi1sb[:, q, :],
                            op0=ALU.mult, op1=ALU.add)
                    ga1 = loop.tile([H, G, BL], fp32, tag="ga1", name="ga1")
                    for q in range(G):
                        nc.scalar.activation(ga1[:, q, :], pw1[:, q, :],
                                             FNS[q], bias=v8[:, 1, q:q + 1],
                                             scale=sv[:, 1, 1, q:q + 1])

                # ---------------- c update ----------------
                ccn = loop.tile([H, 2, BL], fp32, tag="cc", name="ccn")
                if has0:
                    t10 = loop.tile([H, BL], fp32, tag="t10", name="t10")
                    nc.gpsimd.tensor_tensor(t10[:], ga0[:, 1, :], ga0[:, 3, :],
                                            op=ALU.mult)
                    t20 = loop.tile([H, BL], fp32, tag="t20", name="t20")
                    nc.vector.tensor_tensor(t20[:], ga0[:, 0, :], cc[:, 0, :],
                                            op=ALU.mult)
                    nc.vector.tensor_tensor(ccn[:, 0, :], t10[:], t20[:],
                                            op=ALU.add)
                else:
                    nc.gpsimd.tensor_copy(ccn[:, 0, :], cc[:, 0, :])
                if has1:
                    t11 = loop.tile([H, BL], fp32, tag="t11", name="t11")
                    nc.gpsimd.tensor_tensor(t11[:], ga1[:, 1, :], ga1[:, 3, :],
                                            op=ALU.mult)
                    t21 = loop.tile([H, BL], fp32, tag="t21", name="t21")
                    nc.gpsimd.tensor_tensor(t21[:], ga1[:, 0, :], cc[:, 1, :],
                                            op=ALU.mult)
                    nc.gpsimd.tensor_tensor(ccn[:, 1, :], t11[:], t21[:],
                                            op=ALU.add)
                else:
                    nc.vector.tensor_copy(ccn[:, 1, :], cc[:, 1, :])

                # ---------------- AR-B ----------------
                stB = loop.tile([H, 2, 6], fp32, tag="stB", name="stB")
                nc.vector.bn_stats(stB[:, 0, :], ccn[:, 0, :])
                nc.vector.bn_stats(stB[:, 1, :], ccn[:, 1, :])
                brin = loop.tile([H, 4], fp32, tag="brin", name="brin")
                converts(stB[:], brin, 2)
                rbs = allreduce(brin, 4, "arB")

                mvb = loop.tile([H, 4], fp32, tag="mvb", name="mvb")
                nc.vector.tensor_scalar(mvb[:, 0:2], rbs[:, 0:2], MEAN_SCALE,
                                        None, op0=ALU.mult)
                nc.vector.tensor_scalar(mvb[:, 2:4], rbs[:, 2:4], MEAN_SCALE,
                                        EPS, op0=ALU.mult, op1=ALU.add)
                m2b = loop.tile([H, 2], fp32, tag="m2b", name="m2b")
                nc.vector.tensor_tensor(m2b[:], mvb[:, 0:2], mvb[:, 0:2],
                                        op=ALU.mult)
                vepb = loop.tile([H, 2], fp32, tag="vepb", name="vepb")
                nc.vector.scalar_tensor_tensor(
                    vepb[:], in0=m2b[:], scalar=-1.0, in1=mvb[:, 2:4],
                    op0=ALU.mult, op1=ALU.add)
                rsqb = loop.tile([H, 2], fp32, tag="rsqb", name="rsqb")
                quake(rsqb, vepb[:], 2)
                zc = loop.tile([H, 2, BL], fp32, tag="zc", name="zc")
                nc.vector.tensor_tensor(
                    zc[:], ccn[:],
                    mvb[:, 0:2][:, :, None].broadcast_to([H, 2, BL]),
                    op=ALU.subtract)
                scc = loop.tile([H, 2], fp32, tag="scc", name="scc")
                nc.vector.tensor_tensor(scc[:], rsqb[:], gc2_sb[:],
                                        op=ALU.mult)

                # ---------------- h update + wi1 production ----------------
                if has0:
                    tn0 = loop.tile([H, BL], fp32, tag="tn0", name="tn0")
                    nc.scalar.activation(tn0[:], zc[:, 0, :], AF.Tanh,
                                         bias=bec2_sb[:, 0:1],
                                         scale=scc[:, 0:1])
                    h0n = loop.tile([H, BL], mmdt, tag="h0", name="h0n")
                    nc.vector.tensor_tensor(h0n[:], ga0[:, 2, :], tn0[:],
                                            op=ALU.mult)
                    pwm_new = psum.tile([H, G, BL], fp32, tag="w1",
                                        name="pwm")
                    for q in range(G):
                        nc.tensor.matmul(
                            pwm_new[:, q, :],
                            mmcast(wih1_sb[:, q * H:(q + 1) * H]),
                            mmcast(h0n[:]), start=True, stop=True)
                    for q in range(G):
                        nc.vector.bn_stats(stA[:, 12 + q, :],
                                           pwm_new[:, q, :])
                    st_pwm = pwm_new
                    h0 = h0n
                if has1:
                    tn1 = loop.tile([H, BL], fp32, tag="tn1", name="tn1")
                    nc.scalar.activation(tn1[:], zc[:, 1, :], AF.Tanh,
                                         bias=bec2_sb[:, 1:2],
                                         scale=scc[:, 1:2])
                    h1n = loop.tile([H, BL], mmdt, tag="h1", name="h1n")
                    nc.vector.tensor_tensor(h1n[:], ga1[:, 2, :], tn1[:],
                                            op=ALU.mult)
                    h1 = h1n
                cc = ccn
            h1_fin = h1

        # ---------------- final FC ----------------
        for ci in range(2):
            pf = psum.tile([H, O], fp32, tag="w1", name="pf")
            nc.tensor.matmul(pf[:], h1_fin[:, ci * H:(ci + 1) * H],
                             fcw_sb[:], start=True, stop=True)
            yo = loop.tile([H, O], fp32, tag="yo", name="yo")
            nc.scalar.copy(yo[:], pf[:])
            nc.sync.dma_start(
                y[:].rearrange("(c p) o -> c p o", c=2)[ci], yo[:])

    nc.compile()
    return nc


_NC_CACHE = None


def _get_nc():
    global _NC_CACHE
    if _NC_CACHE is None:
        _NC_CACHE = _build_v3()
    return _NC_CACHE


def _prep_inputs(sequences, w_ih0, w_hh0, b0, g_ih0, be_ih0, g_hh0, be_hh0,
                 g_c0, be_c0, w_ih1, w_hh1, b1, g_ih1, be_ih1, g_hh1, be_hh1,
                 g_c1, be_c1, fc_w, fc_b):
    f32 = np.float32

    def pg(v):  # (512,) -> (128, 4)
        return np.ascontiguousarray(np.asarray(v, f32).reshape(G, H).T)

    import ml_dtypes
    bf16 = ml_dtypes.bfloat16
    common = {
        "wih0": np.ascontiguousarray(np.asarray(w_ih0, f32)).astype(bf16),
        "whh0": np.ascontiguousarray(np.asarray(w_hh0, f32)).astype(bf16),
        "wih1": np.ascontiguousarray(np.asarray(w_ih1, f32)).astype(bf16),
        "whh1": np.ascontiguousarray(np.asarray(w_hh1, f32)).astype(bf16),
        "fcwT": np.ascontiguousarray(np.asarray(fc_w, f32).T).astype(bf16),
        "gcat16": np.concatenate(
            [pg(g_hh0), pg(g_ih0), pg(g_hh1), pg(g_ih1)], axis=1).copy(),
        "bccat": np.concatenate(
            [pg(np.asarray(be_hh0) + np.asarray(be_ih0) + np.asarray(b0)),
             pg(np.asarray(be_hh1) + np.asarray(be_ih1) + np.asarray(b1))],
            axis=1).copy(),
        "gc2": np.stack([np.asarray(g_c0, f32),
                         np.asarray(g_c1, f32)], axis=1).copy(),
        "bec2": np.stack([np.asarray(be_c0, f32),
                          np.asarray(be_c1, f32)], axis=1).copy(),
    }
    seq = np.asarray(sequences, f32)
    in_maps = []
    for c in range(NCORES):
        m = dict(common)
        m["xT"] = np.ascontiguousarray(
            seq[c * BL:(c + 1) * BL].transpose(2, 1, 0)).astype(bf16)
        in_maps.append(m)
    return in_maps


def kernel(**inputs):
    nc = _get_nc()
    in_maps = _prep_inputs(**inputs)
    last_exc = None
    for attempt in range(3):
        try:
            res = run_bass_kernel_spmd(nc, in_maps,
                                       core_ids=list(range(NCORES)),
                                       trace=False)
            break
        except Exception as e:
            last_exc = e
            time.sleep(5.0 * (attempt + 1))
    else:
        raise last_exc
    ys = [res.results[c]["y"] for c in range(NCORES)]
    out = np.concatenate(ys, axis=0)
    out = out + np.asarray(inputs["fc_b"], np.float32)[None, :]
    return out.astype(np.float32)


# revision 4
# speedup vs baseline: 1.1422x; 1.1422x over previous
"""BN-LSTM (2-layer, Cooijmans) TRN2 Bass kernel v3 — 8-way batch-parallel.

Key differences vs the v2 baseline (see kernel.py docstring):
- ONE fused loop: no separate input-projection phase. wi0[t] = x_t @ w_ih0 is
  recomputed per step into PSUM (cheap matmul) instead of a 512KB/step DRAM
  round trip; its BN stats ride the same per-step collective as wh0's.
- AllReduce of raw [sum, sumsq] (16KB) instead of AllGather of bn_stats
  records (36KB in / 295KB readback per step) — kills the dominant DMA cost.
- rsqrt via the quake bit-trick + 2 Newton iterations on the Vector engine:
  the Scalar engine runs ONLY sigmoid/tanh/identity (one activation-table
  set) — no ~2.7us table reloads on the critical path.
- wi1[t] stays in PSUM across one step (no scalar copy, no SBUF staging).
- gpsimd queue carries only collectives + their bounce DMAs.
"""

import time
from contextlib import ExitStack

import numpy as np

import concourse.bass as bass
import concourse.mybir as mybir
import concourse.bacc as bacc
import concourse.tile as tile
from concourse.bass_utils import run_bass_kernel_spmd

B, T, I, H, O = 2048, 152, 75, 128, 256
NCORES = 8
BL = B // NCORES
G = 4
EPS = 1e-5

fp32 = mybir.dt.float32
i32 = mybir.dt.int32
AF = mybir.ActivationFunctionType
ALU = mybir.AluOpType
AX = mybir.AxisListType
RG = [list(range(NCORES))]
MAGIC = 0x5F3759DF


def _build_v3(repeats=1, local_stats=False, nr_iters=1, use_f32r=False,
              use_bf16=True):
    nc = bacc.Bacc("TRN2", target_bir_lowering=False, debug=False,
                   num_devices=NCORES)

    mmdt = mybir.dt.bfloat16 if use_bf16 else fp32
    xT = nc.dram_tensor("xT", [I, T, BL], mmdt, kind="ExternalInput").ap()
    wih0 = nc.dram_tensor("wih0", [I, G * H], mmdt, kind="ExternalInput").ap()
    whh0 = nc.dram_tensor("whh0", [H, G * H], mmdt, kind="ExternalInput").ap()
    wih1 = nc.dram_tensor("wih1", [H, G * H], mmdt, kind="ExternalInput").ap()
    whh1 = nc.dram_tensor("whh1", [H, G * H], mmdt, kind="ExternalInput").ap()
    fcwT = nc.dram_tensor("fcwT", [H, O], mmdt, kind="ExternalInput").ap()
    gcat16 = nc.dram_tensor("gcat16", [H, 16], fp32, kind="ExternalInput").ap()
    bccat = nc.dram_tensor("bccat", [H, 8], fp32, kind="ExternalInput").ap()
    gc2 = nc.dram_tensor("gc2", [H, 2], fp32, kind="ExternalInput").ap()
    bec2 = nc.dram_tensor("bec2", [H, 2], fp32, kind="ExternalInput").ap()
    y = nc.dram_tensor("y", [BL, O], fp32, kind="ExternalOutput").ap()

    def mmcast(ap):
        return ap.bitcast(mybir.dt.float32r) if use_f32r else ap

    with tile.TileContext(nc) as tc, ExitStack() as ctx:
        sb = ctx.enter_context(tc.tile_pool(name="sb", bufs=1))
        loop = ctx.enter_context(tc.tile_pool(name="loop", bufs=3))
        psum = ctx.enter_context(tc.tile_pool(name="psum", bufs=1,
                                              space="PSUM"))
        dram = ctx.enter_context(tc.tile_pool(name="dram", bufs=2,
                                              space="DRAM"))

        def load(ap_in, shape, name, dt=fp32):
            t_ = sb.tile(shape, dt, name=name)
            nc.sync.dma_start(t_[:], ap_in[:])
            return t_

        wih0_sb = load(wih0, [I, G * H], "wih0_sb", mmdt)
        whh0_sb = load(whh0, [H, G * H], "whh0_sb", mmdt)
        wih1_sb = load(wih1, [H, G * H], "wih1_sb", mmdt)
        whh1_sb = load(whh1, [H, G * H], "whh1_sb", mmdt)
        fcw_sb = load(fcwT, [H, O], "fcw_sb", mmdt)
        gcat = load(gcat16, [H, 16], "gcat_sb")      # [ghh0|gih0|ghh1|gih1]
        bcc = load(bccat, [H, 8], "bcc_sb")          # [bhh0+bih0+b0 | ..1]
        gc2_sb = load(gc2, [H, 2], "gc2_sb")
        bec2_sb = load(bec2, [H, 2], "bec2_sb")

        magic_t = sb.tile([H, 16], i32, name="magic_t")
        nc.vector.memset(magic_t[:], MAGIC)

        # persistent stats-record tile: 16 groups x 6 bn_stats fields
        # group layout: [wh0 0:4 | wi0 4:8 | wh1 8:12 | wi1 12:16]
        stA = sb.tile([H, 16, 6], fp32, name="stA")
        nc.vector.memset(stA[:], 0.0)

        def quake(dst, src, n, eng=nc.vector, nr=nr_iters):
            """dst = 1/sqrt(src) elementwise, (H, n) tiles, DVE-only."""
            di = dst[:].bitcast(i32)
            eng.tensor_scalar(di, src.bitcast(i32), 1, None,
                              op0=ALU.logical_shift_right)
            eng.tensor_tensor(di, magic_t[:, 0:n].bitcast(i32), di,
                              op=ALU.subtract)
            for _ in range(nr):
                yy = loop.tile([H, n], fp32, tag=f"qk_yy{n}", name="yy")
                eng.tensor_tensor(yy[:], dst[:], dst[:], op=ALU.mult)
                eng.tensor_tensor(yy[:], src, yy[:], op=ALU.mult)
                eng.tensor_scalar(yy[:], yy[:], -0.5, 1.5, op0=ALU.mult,
                                  op1=ALU.add)
                eng.tensor_tensor(dst[:], dst[:], yy[:], op=ALU.mult)

        def converts(st_view, out_tile, n, eng=nc.gpsimd):
            """bn_stats records (H, n, 6) -> out (H, 2n) = [128*(me+mo) sums,
            sumsq]."""
            me = st_view[:, :, 1]
            mo = st_view[:, :, 4]
            s1 = loop.tile([H, n], fp32, tag=f"cv_s1{n}", name="s1")
            eng.tensor_tensor(s1[:], me, mo, op=ALU.add)
            nc.vector.tensor_scalar(out_tile[:, 0:n], s1[:], float(BL // 2),
                                    None, op0=ALU.mult)
            q1 = loop.tile([H, n], fp32, tag=f"cv_q1{n}", name="q1")
            eng.tensor_tensor(q1[:], me, me, op=ALU.mult)
            q2 = loop.tile([H, n], fp32, tag=f"cv_q2{n}", name="q2")
            eng.tensor_tensor(q2[:], mo, mo, op=ALU.mult)
            eng.tensor_tensor(q1[:], q1[:], q2[:], op=ALU.add)
            m2s = loop.tile([H, n], fp32, tag=f"cv_m2{n}", name="m2s")
            eng.tensor_tensor(m2s[:], st_view[:, :, 2], st_view[:, :, 5],
                              op=ALU.add)
            nc.vector.scalar_tensor_tensor(
                out_tile[:, n:2 * n], in0=q1[:], scalar=float(BL // 2),
                in1=m2s[:], op0=ALU.mult, op1=ALU.add)

        def allreduce(in_tile, n, tag):
            """AllGather + local 8-way tree add -> SBUF tile (H, n)."""
            din = dram.tile([H, n], fp32, tag=f"{tag}_in", name=f"{tag}_in")
            nc.gpsimd.dma_start(din[:], in_tile[:])
            if local_stats:
                res = loop.tile([H, n], fp32, tag=f"{tag}_res",
                                name=f"{tag}_res")
                nc.gpsimd.dma_start(res[:], din[:])
                # fake the x8 sum so magnitudes match the collective path
                nc.vector.tensor_scalar(res[:], res[:], 8.0, None,
                                        op0=ALU.mult)
                return res
            dout = dram.tile([NCORES * H, n], fp32, tag=f"{tag}_out",
                             addr_space="Shared", name=f"{tag}_out")
            nc.gpsimd.collective_compute(
                "AllGather", ALU.bypass, replica_groups=RG,
                ins=[din[:]], outs=[dout[:]])
            rb = loop.tile([H, NCORES, n], fp32, tag=f"{tag}_rb",
                           name=f"{tag}_rb")
            nc.gpsimd.dma_start(
                rb[:], dout[:].rearrange("(r p) s -> p r s", r=NCORES))
            v4 = rb[:].rearrange("p (a b) s -> p a b s", a=2)
            r4 = loop.tile([H, 4, n], fp32, tag=f"{tag}_r4", name=f"{tag}_r4")
            nc.vector.tensor_tensor(r4[:], v4[:, 0], v4[:, 1], op=ALU.add)
            r4v = r4[:].rearrange("p (a b) s -> p a b s", a=2)
            r2 = loop.tile([H, 2, n], fp32, tag=f"{tag}_r2", name=f"{tag}_r2")
            nc.vector.tensor_tensor(r2[:], r4v[:, 0], r4v[:, 1], op=ALU.add)
            res = loop.tile([H, n], fp32, tag=f"{tag}_res", name=f"{tag}_res")
            nc.vector.tensor_tensor(res[:], r2[:, 0], r2[:, 1], op=ALU.add)
            return res

        h0 = loop.tile([H, BL], mmdt, tag="h0", name="h0")
        h1 = loop.tile([H, BL], mmdt, tag="h1", name="h1")
        cc = loop.tile([H, 2, BL], fp32, tag="cc", name="cc")
        for t_ in (h0, h1):
            nc.vector.memset(t_[:], 0.0)
        nc.vector.memset(cc[:], 0.0)

        scale_mean = 1.0 / (B // (BL // 2))  # AR(sum)/2048 with the 128 factor
        # arin[:,0:16] already scaled by BL/2=128, so mean = AR/(2048/1)*... :
        # arin holds 128*(me+mo) = sum. So mean factor is 1/2048.
        MEAN_SCALE = 1.0 / B

        h1_fin = None
        for _rep in range(repeats):
            if _rep > 0:
                for t_ in (h0, h1):
                    nc.vector.memset(t_[:], 0.0)
                nc.vector.memset(cc[:], 0.0)
                nc.vector.memset(stA[:], 0.0)

            for t in range(T + 1):
                has0 = t < T
                has1 = t >= 1

                # ---------------- matmuls + stats ----------------
                pw0 = pwi0 = pw1 = None
                if has0:
                    xt = loop.tile([I, BL], mmdt, tag="xt", bufs=4, name="xt")
                    nc.sync.dma_start(xt[:], xT[:, t, :])
                    pwi0 = psum.tile([H, G, BL], fp32, tag="wi0", name="pwi0")
                    for q in range(G):
                        nc.tensor.matmul(
                            pwi0[:, q, :],
                            mmcast(wih0_sb[:, q * H:(q + 1) * H]),
                            mmcast(xt[:]), start=True, stop=True)
                    pw0 = psum.tile([H, G, BL], fp32, tag="g0", name="pw0")
                    for q in range(G):
                        nc.tensor.matmul(
                            pw0[:, q, :],
                            mmcast(whh0_sb[:, q * H:(q + 1) * H]),
                            mmcast(h0[:]), start=True, stop=True)
                    for q in range(G):
                        nc.vector.bn_stats(stA[:, q, :], pw0[:, q, :])
                        nc.vector.bn_stats(stA[:, 4 + q, :], pwi0[:, q, :])
                if has1:
                    pw1 = psum.tile([H, G, BL], fp32, tag="g1", name="pw1")
                    for q in range(G):
                        nc.tensor.matmul(
                            pw1[:, q, :],
                            mmcast(whh1_sb[:, q * H:(q + 1) * H]),
                            mmcast(h1[:]), start=True, stop=True)
                    for q in range(G):
                        nc.vector.bn_stats(stA[:, 8 + q, :], pw1[:, q, :])
                    # stA[:, 12:16] (wi1[t-1]) written at tail of step t-1

                # ---------------- AR-A ----------------
                arin = loop.tile([H, 32], fp32, tag="arin", name="arin")
                converts(stA[:], arin, 16)
                ra = allreduce(arin, 32, "arA")
                if has0:
                    # PSUM->SBUF copy runs during the collective wait
                    wi0sb = loop.tile([H, G, BL], fp32, tag="wi0sb",
                                      name="wi0sb")
                    nc.scalar.copy(wi0sb[:], pwi0[:])
                if has1:
                    st_wi1sb = loop.tile([H, G, BL], fp32, tag="wi1sb",
                                         name="wi1sb")
                    nc.scalar.copy(st_wi1sb[:], st_pwm[:])

                # ---------------- post-AR-A scales ----------------
                mv = loop.tile([H, 32], fp32, tag="mv", name="mv")
                nc.vector.tensor_scalar(mv[:, 0:16], ra[:, 0:16], MEAN_SCALE,
                                        None, op0=ALU.mult)
                # E2 + eps in one op; var+eps = (E2+eps) - mean^2
                nc.vector.tensor_scalar(mv[:, 16:32], ra[:, 16:32], MEAN_SCALE,
                                        EPS, op0=ALU.mult, op1=ALU.add)
                m2 = loop.tile([H, 16], fp32, tag="m2", name="m2")
                nc.vector.tensor_tensor(m2[:], mv[:, 0:16], mv[:, 0:16],
                                        op=ALU.mult)
                vep = loop.tile([H, 16], fp32, tag="vep", name="vep")
                nc.vector.scalar_tensor_tensor(
                    vep[:], in0=m2[:], scalar=-1.0, in1=mv[:, 16:32],
                    op0=ALU.mult, op1=ALU.add)
                rsq = loop.tile([H, 16], fp32, tag="rsq", name="rsq")
                quake(rsq, vep[:], 16)
                s16 = loop.tile([H, 16], fp32, tag="s16", name="s16")
                nc.vector.tensor_tensor(s16[:], rsq[:], gcat[:], op=ALU.mult)
                ms = loop.tile([H, 16], fp32, tag="ms", name="ms")
                nc.vector.tensor_tensor(ms[:], mv[:, 0:16], s16[:],
                                        op=ALU.mult)
                # layered views: (H, 2, 4): [:,0,:]=layer0, [:,1,:]=layer1
                sv = s16[:].rearrange("p (l k g) -> p l k g", l=2, k=2)
                msv = ms[:].rearrange("p (l k g) -> p l k g", l=2, k=2)
                is8 = loop.tile([H, 2, G], fp32, tag="is8", name="is8")
                nc.vector.reciprocal(is8[:], sv[:, :, 1, :])
                r8 = loop.tile([H, 2, G], fp32, tag="r8", name="r8")
                nc.vector.tensor_tensor(r8[:], sv[:, :, 0, :], is8[:],
                                        op=ALU.mult)
                v8 = loop.tile([H, 2, G], fp32, tag="v8", name="v8")
                bcv = bcc[:].rearrange("p (l g) -> p l g", l=2)
                nc.vector.tensor_tensor(v8[:], bcv, msv[:, :, 0, :],
                                        op=ALU.subtract)
                nc.vector.tensor_tensor(v8[:], v8[:], msv[:, :, 1, :],
                                        op=ALU.subtract)

                # ---------------- gates ----------------
                FNS = (AF.Sigmoid, AF.Sigmoid, AF.Sigmoid, AF.Tanh)
                ga0 = ga1 = None
                if has0:
                    for q in range(G):
                        nc.vector.scalar_tensor_tensor(
                            pw0[:, q, :], in0=pw0[:, q, :],
                            scalar=r8[:, 0, q:q + 1], in1=wi0sb[:, q, :],
                            op0=ALU.mult, op1=ALU.add)
                    ga0 = loop.tile([H, G, BL], fp32, tag="ga0", name="ga0")
                    for q in range(G):
                        nc.scalar.activation(ga0[:, q, :], pw0[:, q, :],
                                             FNS[q], bias=v8[:, 0, q:q + 1],
                                             scale=sv[:, 0, 1, q:q + 1])
                if has1:
                    for q in range(G):
                        nc.vector.scalar_tensor_tensor(
                            pw1[:, q, :], in0=pw1[:, q, :],
                            scalar=r8[:, 1, q:q + 1], in1=st_wi1sb[:, q, :],
                            op0=ALU.mult, op1=ALU.add)
                    ga1 = loop.tile([H, G, BL], fp32, tag="ga1", name="ga1")
                    for q in range(G):
                        nc.scalar.activation(ga1[:, q, :], pw1[:, q, :],
                                             FNS[q], bias=v8[:, 1, q:q + 1],
                                             scale=sv[:, 1, 1, q:q + 1])

                # ---------------- c update ----------------
                ccn = loop.tile([H, 2, BL], fp32, tag="cc", name="ccn")
                if has0:
                    t10 = loop.tile([H, BL], fp32, tag="t10", name="t10")
                    nc.gpsimd.tensor_tensor(t10[:], ga0[:, 1, :], ga0[:, 3, :],
                                            op=ALU.mult)
                    t20 = loop.tile([H, BL], fp32, tag="t20", name="t20")
                    nc.vector.tensor_tensor(t20[:], ga0[:, 0, :], cc[:, 0, :],
                                            op=ALU.mult)
                    nc.vector.tensor_tensor(ccn[:, 0, :], t10[:], t20[:],
                                            op=ALU.add)
                else:
                    nc.gpsimd.tensor_copy(ccn[:, 0, :], cc[:, 0, :])
                if has1:
                    t11 = loop.tile([H, BL], fp32, tag="t11", name="t11")
                    nc.gpsimd.tensor_tensor(t11[:], ga1[:, 1, :], ga1[:, 3, :],
                                            op=ALU.mult)
                    t21 = loop.tile([H, BL], fp32, tag="t21", name="t21")
                    nc.gpsimd.tensor_tensor(t21[:], ga1[:, 0, :], cc[:, 1, :],
                                            op=ALU.mult)
                    nc.gpsimd.tensor_tensor(ccn[:, 1, :], t11[:], t21[:],
                                            op=ALU.add)
                else:
                    nc.vector.tensor_copy(ccn[:, 1, :], cc[:, 1, :])

                # ---------------- AR-B ----------------
                stB = loop.tile([H, 2, 6], fp32, tag="stB", name="stB")
                nc.vector.bn_stats(stB[:, 0, :], ccn[:, 0, :])
                nc.vector.bn_stats(stB[:, 1, :], ccn[:, 1, :])
                brin = loop.tile([H, 4], fp32, tag="brin", name="brin")
                converts(stB[:], brin, 2)
                rbs = allreduce(brin, 4, "arB")

                mvb = loop.tile([H, 4], fp32, tag="mvb", name="mvb")
                nc.vector.tensor_scalar(mvb[:, 0:2], rbs[:, 0:2], MEAN_SCALE,
                                        None, op0=ALU.mult)
                nc.vector.tensor_scalar(mvb[:, 2:4], rbs[:, 2:4], MEAN_SCALE,
                                        EPS, op0=ALU.mult, op1=ALU.add)
                m2b = loop.tile([H, 2], fp32, tag="m2b", name="m2b")
                nc.vector.tensor_tensor(m2b[:], mvb[:, 0:2], mvb[:, 0:2],
                                        op=ALU.mult)
                vepb = loop.tile([H, 2], fp32, tag="vepb", name="vepb")
                nc.vector.scalar_tensor_tensor(
                    vepb[:], in0=m2b[:], scalar=-1.0, in1=mvb[:, 2:4],
                    op0=ALU.mult, op1=ALU.add)
                rsqb = loop.tile([H, 2], fp32, tag="rsqb", name="rsqb")
                quake(rsqb, vepb[:], 2)
                zc = loop.tile([H, 2, BL], fp32, tag="zc", name="zc")
                nc.vector.tensor_tensor(
                    zc[:], ccn[:],
                    mvb[:, 0:2][:, :, None].broadcast_to([H, 2, BL]),
                    op=ALU.subtract)
                scc = loop.tile([H, 2], fp32, tag="scc", name="scc")
                nc.vector.tensor_tensor(scc[:], rsqb[:], gc2_sb[:],
                                        op=ALU.mult)

                # ---------------- h update + wi1 production ----------------
                if has0:
                    tn0 = loop.tile([H, BL], fp32, tag="tn0", name="tn0")
                    nc.scalar.activation(tn0[:], zc[:, 0, :], AF.Tanh,
                                         bias=bec2_sb[:, 0:1],
                                         scale=scc[:, 0:1])
                    h0n = loop.tile([H, BL], mmdt, tag="h0", name="h0n")
                    nc.vector.tensor_tensor(h0n[:], ga0[:, 2, :], tn0[:],
                                            op=ALU.mult)
                    pwm_new = psum.tile([H, G, BL], fp32, tag="w1",
                                        name="pwm")
                    for q in range(G):
                        nc.tensor.matmul(
                            pwm_new[:, q, :],
                            mmcast(wih1_sb[:, q * H:(q + 1) * H]),
                            mmcast(h0n[:]), start=True, stop=True)
                    for q in range(G):
                        nc.vector.bn_stats(stA[:, 12 + q, :],
                                           pwm_new[:, q, :])
                    st_pwm = pwm_new
                    h0 = h0n
                if has1:
                    tn1 = loop.tile([H, BL], fp32, tag="tn1", name="tn1")
                    nc.scalar.activation(tn1[:], zc[:, 1, :], AF.Tanh,
                                         bias=bec2_sb[:, 1:2],
                                         scale=scc[:, 1:2])
                    h1n = loop.tile([H, BL], mmdt, tag="h1", name="h1n")
                    nc.vector.tensor_tensor(h1n[:], ga1[:, 2, :], tn1[:],
                                            op=ALU.mult)
                    h1 = h1n
                cc = ccn
            h1_fin = h1

        # ---------------- final FC ----------------
        for ci in range(2):
            pf = psum.tile([H, O], fp32, tag="w1", name="pf")
            nc.tensor.matmul(pf[:], h1_fin[:, ci * H:(ci + 1) * H],
                             fcw_sb[:], start=True, stop=True)
            yo = loop.tile([H, O], fp32, tag="yo", name="yo")
            nc.scalar.copy(yo[:], pf[:])
            nc.sync.dma_start(
                y[:].rearrange("(c p) o -> c p o", c=2)[ci], yo[:])

    nc.compile()
    return nc


_NC_CACHE = None


def _get_nc():
    global _NC_CACHE
    if _NC_CACHE is None:
        _NC_CACHE = _build_v3()
    return _NC_CACHE


def _prep_inputs(sequences, w_ih0, w_hh0, b0, g_ih0, be_ih0, g_hh0, be_hh0,
                 g_c0, be_c0, w_ih1, w_hh1, b1, g_ih1, be_ih1, g_hh1, be_hh1,
                 g_c1, be_c1, fc_w, fc_b):
    f32 = np.float32

    def pg(v):  # (512,) -> (128, 4)
        return np.ascontiguousarray(np.asarray(v, f32).reshape(G, H).T)

    import ml_dtypes
    bf16 = ml_dtypes.bfloat16
    common = {
        "wih0": np.ascontiguousarray(np.asarray(w_ih0, f32)).astype(bf16),
        "whh0": np.ascontiguousarray(np.asarray(w_hh0, f32)).astype(bf16),
        "wih1": np.ascontiguousarray(np.asarray(w_ih1, f32)).astype(bf16),
        "whh1": np.ascontiguousarray(np.asarray(w_hh1, f32)).astype(bf16),
        "fcwT": np.ascontiguousarray(np.asarray(fc_w, f32).T).astype(bf16),
        "gcat16": np.concatenate(
            [pg(g_hh0), pg(g_ih0), pg(g_hh1), pg(g_ih1)], axis=1).copy(),
        "bccat": np.concatenate(
            [pg(np.asarray(be_hh0) + np.asarray(be_ih0) + np.asarray(b0)),
             pg(np.asarray(be_hh1) + np.asarray(be_ih1) + np.asarray(b1))],
            axis=1).copy(),
        "gc2": np.stack([np.asarray(g_c0, f32),
                         np.asarray(g_c1, f32)], axis=1).copy(),
        "bec2": np.stack([np.asarray(be_c0, f32),
                          np.asarray(be_c1, f32)], axis=1).copy(),
    }
    seq = np.asarray(sequences, f32)
    in_maps = []
    for c in range(NCORES):
        m = dict(common)
        m["xT"] = np.ascontiguousarray(
            seq[c * BL:(c + 1) * BL].transpose(2, 1, 0)).astype(bf16)
        in_maps.append(m)
    return in_maps


def kernel(**inputs):
    nc = _get_nc()
    in_maps = _prep_inputs(**inputs)
    last_exc = None
    for attempt in range(3):
        try:
            res = run_bass_kernel_spmd(nc, in_maps,
                                       core_ids=list(range(NCORES)),
                                       trace=False)
            break
        except Exception as e:
            last_exc = e
            time.sleep(5.0 * (attempt + 1))
    else:
        raise last_exc
    ys = [res.results[c]["y"] for c in range(NCORES)]
    out = np.concatenate(ys, axis=0)
    out = out + np.asarray(inputs["fc_b"], np.float32)[None, :]
    return out.astype(np.float32)
